# revision 12
# baseline (speedup 1.0000x reference)
"""Trainium2 Bass kernel for single-head attention (nn_MultiHeadAttention).

Reference computation (B=4, S=2048, D=1024, fp32):
    K = _K @ Wk.T + bk ; V = _V @ Wv.T + bv ; Q = _Q @ Wq.T + bq
    scores[b,k,q] = (K[b,k,:] . Q[b,q,:]) / sqrt(D)
    alpha = softmax(scores, axis=keys)
    V_[b,q,:] = sum_k V[b,k,:] * alpha[b,k,q]
    O = V_ @ Wo.T + bo

Sharding: core c = (b, h) with b = c//2 (batch), h = c%2 (query half of
1024). Each core handles the full key/value sequence of its batch and a
1024-query slice. Cores {2b, 2b+1} share batch b: each projects half the
keys/values and the halves are exchanged pair-wise with an AllGather.

Structural choices (all aimed at zero PE idle — the HAM clock gate
re-throttles the array from 2.4 to 1.2 GHz after ~3.4us idle):
  - Own-half-in-place exchange: each core's projected half goes straight
    to its final SBUF slot (local key order = [own half; partner half];
    softmax and the V-weighted sum are key-order invariant so any local
    order is valid). The pair AllGather output holds [rank0; rank1]; the
    partner half is recovered rank-agnostically as
    (r0 - own) + r1 with an fp32 intermediate — exact, because one of
    r0/r1 is bit-identical to own. Phase B scores own-half keys while
    the collective is still in flight.
  - All input streams are loaded as [128, 1024] tiles (2KB per partition
    line — small descriptors measurably halve effective DMA bandwidth)
    and are fully resident before the collective window opens, so the
    collective's HBM traffic never contends with the input streams.
  - DMA queue split: input streams + gather-back loads on the Sync DGE
    queue; collective staging stores + output stores on the Activation
    DGE queue.
  - A burst of dummy matmuls fed from the first weight block (no memset
    dependency — the Vector engine's start preamble is ~7us) warms the
    HAM clock gate before real work; a tiny AllGather issued first
    absorbs the one-time comm-init barrier.
  - Key-sums for softmax: exp tiles accumulate on GpSimd (otherwise
    idle), reduced across partitions at the end with one fp32
    ones-matmul per query chunk instead of 32 per-block ones-matmuls.
All main matmuls are bf16 (M=128, N=512) accumulating in fp32 PSUM.
"""

import sys

if "/opt/trn_rl_repo" not in sys.path:
    sys.path.insert(0, "/opt/trn_rl_repo")

import ml_dtypes
import numpy as np

import concourse.bass as bass
import concourse.tile as tile
from concourse import bacc, mybir
from concourse.bass_utils import run_bass_kernel_spmd

B, S, D = 4, 2048, 1024
SQ = 1024  # queries per core
SH = 1024  # keys/values projected per core (pair exchange fills the rest)
P = 128  # partitions
CH = 512  # matmul moving free dim (one fp32 PSUM bank)
EB = D // P  # 8 feature blocks
DB = D // P  # 8 contraction blocks
KB = S // P  # 16 key blocks
QB = SQ // P  # 8 query blocks
QC = SQ // CH  # 2 query chunks
FC = D // CH  # 2 output-feature chunks
SCALE = 1.0 / np.sqrt(np.float32(D))  # folded into exp()

F32 = mybir.dt.float32
BF16 = mybir.dt.bfloat16
AF = mybir.ActivationFunctionType
NPBF16 = ml_dtypes.bfloat16

N_WARM_MM = 16  # dummy matmuls to flip the HAM clock gate before real work

# test.py can flip this to get a profiled run; the measured NEFF time (max
# over traced cores) lands in LAST_EXEC_NS.
TRACE = False
TRACE_ALL_CORES = False
LAST_EXEC_NS = None

_NC_CACHE = None


def _build_nc() -> bass.Bass:
    # Bacc (not plain Bass): its finalize() pipeline splits multi-sem waits
    # into event-semaphore chains — TRN2 instructions take at most 1 wait.
    nc = bacc.Bacc(num_devices=8)

    kt_d = nc.dram_tensor("kt", [D, SH], BF16, kind="ExternalInput")
    vt_d = nc.dram_tensor("vt", [D, SH], BF16, kind="ExternalInput")
    qt_d = nc.dram_tensor("qt", [D, SQ], BF16, kind="ExternalInput")
    wkt_d = nc.dram_tensor("wkt", [D, D], BF16, kind="ExternalInput")
    wqt_d = nc.dram_tensor("wqt", [D, D], BF16, kind="ExternalInput")
    wvt_d = nc.dram_tensor("wvt", [D, D], BF16, kind="ExternalInput")
    wot_d = nc.dram_tensor("wot", [D, D], BF16, kind="ExternalInput")
    bk_d = nc.dram_tensor("bk", [P, EB], F32, kind="ExternalInput")
    bq_d = nc.dram_tensor("bq", [P, EB], F32, kind="ExternalInput")
    bvb_d = nc.dram_tensor("bvb", [P, D], BF16, kind="ExternalInput")
    bob_d = nc.dram_tensor("bob", [P, D], BF16, kind="ExternalInput")
    o_d = nc.dram_tensor("o", [SQ, D], F32, kind="ExternalOutput")

    with tile.TileContext(nc) as tc:
        # Pools are stack-allocated per SBUF side. The weight ring (wa,
        # bufs=2) cycles wkt->wqt->wvt->wot through two 16KB slots; each
        # reuse WAR-waits only on the previous phase's matmuls, which are
        # done long before the next weight is needed. kt/qt (kqt) are
        # ACT/DVE-written only, so that region is safely recycled for vtu
        # once phase B is done.
        p_misc = tc.alloc_tile_pool(name="misc", bufs=1, side="left")
        p_ps = tc.alloc_tile_pool(name="ps", bufs=6, space="PSUM")
        p_pss = tc.alloc_tile_pool(name="pss", bufs=2, space="PSUM")
        p_v = tc.alloc_tile_pool(name="v", bufs=1, side="right")
        p_xs = tc.alloc_tile_pool(name="xs", bufs=8, side="right")
        p_vs = tc.alloc_tile_pool(name="vs", bufs=8, side="right")
        p_cs = tc.alloc_tile_pool(name="cs", bufs=4, side="right")
        p_tmp = tc.alloc_tile_pool(name="tmp", bufs=2, side="right")
        p_wa = tc.alloc_tile_pool(name="wa", bufs=2, side="left")
        p_kqt = tc.alloc_tile_pool(name="kqt", bufs=1, side="left")

        p_dram = tc.alloc_tile_pool(name="dram", bufs=1, space="DRAM")

        dma = nc.sync.dma_start
        dma_act = nc.scalar.dma_start

        recip_sb = p_misc.tile([P, SQ], BF16)
        ones32_sb = p_misc.tile([P, P], F32)  # key-sum partition reduction
        acc_sb = p_misc.tile([P, QC, CH], F32)  # exp-sum accumulators
        nc.vector.memset(ones32_sb[:], 1.0)

        # Pair-wise exchange groups: {2b, 2b+1} share batch b.
        # The first collective pays a large one-time comm-init cost, so a
        # 128-byte warmup AllGather is issued immediately (staged via the
        # fast Sync DGE — the gpsimd software DGE takes ~10us to build
        # descriptors) and initializes the channels during phase A.
        CC_GROUPS = [[0, 1], [2, 3], [4, 5], [6, 7]]
        warm_in = p_dram.tile([1, 64], BF16)
        warm_out = p_dram.tile([2, 64], BF16)
        dma(out=warm_in[:], in_=kt_d[0:1, 0:64])
        nc.gpsimd.collective_compute(
            "AllGather",
            mybir.AluOpType.bypass,
            replica_groups=CC_GROUPS,
            ins=[warm_in.opt()],
            outs=[warm_out.opt()],
        )
        cc_kin = p_dram.tile([D, SH], BF16)
        cc_kout = p_dram.tile([2 * D, SH], BF16)
        cc_vin = p_dram.tile([SH, D], BF16)
        cc_vout = p_dram.tile([2 * SH, D], BF16)

        # One DMA per d-block so loads spread across HW queues and each
        # matmul depends only on its own 256KB slice.
        def load_w(name):
            t = p_wa.tile([P, DB, D], BF16, tag="w", name=name)
            src = {"wkt": wkt_d, "wqt": wqt_d, "wvt": wvt_d, "wot": wot_d}[
                name
            ].rearrange("(a p) e -> p a e", p=P)
            for a in range(DB):
                dma(out=t[:, a, :], in_=src[:, a, :])
            return t

        wkt_sb = load_w("wkt")
        bk_sb = p_misc.tile([P, EB], F32)
        dma(out=bk_sb[:], in_=bk_d[:])
        bq_sb = p_misc.tile([P, EB], F32)
        dma(out=bq_sb[:], in_=bq_d[:])

        # ---- PE warmup: flip the HAM clock gate to 2.4 GHz while the
        # input DMAs are still in flight. The dummy matmuls read the first
        # weight block (lands ~3us — a memset operand would be gated on
        # the Vector engine's ~7us start preamble) and write a scratch
        # PSUM slot recycled by the first projection group.
        warm_ps = p_ps.tile([P, CH], F32, tag="ps", name="ps")
        for _ in range(N_WARM_MM):
            nc.tensor.matmul(
                warm_ps[:, 0:P],
                wkt_sb[:, 0, 0:P],
                wkt_sb[:, 0, 0:P],
                start=True,
                stop=True,
            )

        kt_sb = p_kqt.tile([P, EB, S], BF16)  # K.T: [e_p, e_blk, k]
        qt_sb = p_kqt.tile([P, EB, SQ], BF16)  # Q.T: [e_p, e_blk, q]
        v_sb = p_v.tile([P, KB, D], BF16)  # V:   [k_p, k_blk, e]

        # ---- Phase A: projections ----
        # Q.T and K.T: out[e, s] = sum_d W.T[d, e] (stationary) @ _X.T[d, s]
        # Input streams are [128, 1024] tiles; the matmuls slice the moving
        # operand per 512-chunk. The K stream rides the Activation DGE
        # queue so it lands in parallel with the weights on the Sync queue.
        def kq_proj(proj_w, proj_in, proj_out, proj_b, nchunk, dma_x=dma):
            xtt = []
            for d_ in range(DB):
                t = p_xs.tile([P, SH], BF16, tag="xtt", name="xtt")
                dma_x(out=t[:], in_=proj_in[d_ * P : (d_ + 1) * P, :])
                xtt.append(t)
            for sc in range(nchunk):
                for eb in range(EB):
                    ps = p_ps.tile([P, CH], F32, tag="ps", name="ps")
                    for d_ in range(DB):
                        nc.tensor.matmul(
                            ps[:],
                            proj_w[:, d_, eb * P : (eb + 1) * P],
                            xtt[d_][:, sc * CH : (sc + 1) * CH],
                            start=(d_ == 0),
                            stop=(d_ == DB - 1),
                        )
                    # DVE, not ACT: ~3x faster per copy-out, frees the psum
                    # slot sooner, and keeps ScalarE clear for phase B's exp
                    nc.vector.tensor_scalar_add(
                        proj_out[:, eb, sc * CH : (sc + 1) * CH],
                        ps[:],
                        proj_b[:, eb : eb + 1],
                    )

        # K.T own half lands directly in its final slot kt_sb[:, :, 0:SH].
        kq_proj(wkt_sb, kt_d, kt_sb, bk_sb, SH // CH, dma_x=dma_act)
        for eb in range(EB):
            dma_act(out=cc_kin[eb * P : (eb + 1) * P, :], in_=kt_sb[:, eb, 0:SH])
        nc.gpsimd.collective_compute(
            "AllGather",
            mybir.AluOpType.bypass,
            replica_groups=CC_GROUPS,
            ins=[cc_kin.opt()],
            outs=[cc_kout.opt()],
        )

        wvt_sb = load_w("wvt")
        bvb_sb = p_misc.tile([P, D], BF16)
        dma(out=bvb_sb[:], in_=bvb_d[:])

        # V natural: out[k, e] = sum_d _V.T[d, k] (stationary) @ Wv.T[d, e]
        vtb = []
        for d_ in range(DB):
            t = p_vs.tile([P, SH], BF16, tag="vtt", name="vtt")
            dma(out=t[:], in_=vt_d[d_ * P : (d_ + 1) * P, :])
            vtb.append(t)
        for kb in range(SH // P):
            pse = [p_ps.tile([P, CH], F32, tag="ps", name="ps") for _ in range(FC)]
            for d_ in range(DB):
                for eh in range(FC):
                    nc.tensor.matmul(
                        pse[eh][:],
                        vtb[d_][:, kb * P : (kb + 1) * P],
                        wvt_sb[:, d_, eh * CH : (eh + 1) * CH],
                        start=(d_ == 0),
                        stop=(d_ == DB - 1),
                    )
            for eh in range(FC):
                nc.vector.tensor_add(
                    v_sb[:, kb, eh * CH : (eh + 1) * CH],
                    pse[eh][:],
                    bvb_sb[:, eh * CH : (eh + 1) * CH],
                )

        # V pair exchange (own half already in v_sb[:, 0:8, :])
        for kb in range(SH // P):
            dma_act(out=cc_vin[kb * P : (kb + 1) * P, :], in_=v_sb[:, kb, :])
        nc.gpsimd.collective_compute(
            "AllGather",
            mybir.AluOpType.bypass,
            replica_groups=CC_GROUPS,
            ins=[cc_vin.opt()],
            outs=[cc_vout.opt()],
        )

        # Q projection last: its input tiles reuse the K stream's SBUF
        # slots (WAR on the K matmuls, which are long done), and phase B
        # follows it seamlessly on the PE.
        wqt_sb = load_w("wqt")
        kq_proj(wqt_sb, qt_d, qt_sb, bq_sb, QC)

        wot_sb = load_w("wot")
        bob_sb = p_misc.tile([P, D], BF16)
        dma(out=bob_sb[:], in_=bob_d[:])

        # Partner halves: partner = (r0 - own) + r1. One of r0/r1 is
        # bit-identical to own, so with the fp32 intermediate (512-wide
        # halves to keep the ring small) the recovery is exact on both
        # ranks.
        def recover(own_ap, out_ap, cc_out, row0):
            la = p_cs.tile([P, SH], BF16, tag="cs", name="cs")
            dma(out=la[:], in_=cc_out[row0 : row0 + P, :])
            lb = p_cs.tile([P, SH], BF16, tag="cs", name="cs")
            dma(out=lb[:], in_=cc_out[row0 + SH : row0 + SH + P, :])
            for hh in range(2):
                sl = slice(hh * CH, (hh + 1) * CH)
                t32 = p_tmp.tile([P, CH], F32, tag="t32", name="t32")
                nc.vector.tensor_sub(t32[:], la[:, sl], own_ap[:, sl])
                nc.vector.tensor_add(out_ap[:, sl], t32[:], lb[:, sl])

        for eb in range(EB):
            recover(kt_sb[:, eb, 0:SH], kt_sb[:, eb, SH:S], cc_kout, eb * P)
        for kb in range(SH // P):
            recover(v_sb[:, kb, :], v_sb[:, 8 + kb, :], cc_vout, kb * P)

        p_es = tc.alloc_tile_pool(name="es", bufs=1, side="right")
        es_sb = p_es.tile([P, KB, SQ], BF16)  # exp(scores): [k_p, k_blk, q]

        # ---- Phase B: scores[k, q] = K.T' @ Q.T, exp, and key-sums ----
        # Key blocks 0..7 are the own half (ready right after the K
        # projection); 8..15 are the partner half (recovered well before
        # the PE's in-order queue reaches them). exp tiles accumulate on
        # GpSimd; the partition reduction happens once at the end with a
        # single fp32 ones-matmul per query chunk.
        nc.gpsimd.memset(acc_sb[:], 0.0)
        for kb in range(KB):
            psq = [p_ps.tile([P, CH], F32, tag="ps", name="ps") for _ in range(QC)]
            for eb in range(EB):
                for qc in range(QC):
                    nc.tensor.matmul(
                        psq[qc][:],
                        kt_sb[:, eb, kb * P : (kb + 1) * P],
                        qt_sb[:, eb, qc * CH : (qc + 1) * CH],
                        start=(eb == 0),
                        stop=(eb == EB - 1),
                    )
            for qc in range(QC):
                nc.scalar.activation(
                    es_sb[:, kb, qc * CH : (qc + 1) * CH],
                    psq[qc][:],
                    AF.Exp,
                    scale=float(SCALE),
                )
                nc.gpsimd.tensor_add(
                    acc_sb[:, qc, :],
                    acc_sb[:, qc, :],
                    es_sb[:, kb, qc * CH : (qc + 1) * CH],
                )

        p_kqt.release()
        p_vtu = tc.alloc_tile_pool(name="vtu", bufs=1, side="left")
        vtu_sb = p_vtu.tile([P, EB, SQ], BF16)  # normalized V_.T: [e_p, e_blk, q]

        # ---- Phase C: V_.T[e, q] = (sum_k V[k, e] es[k, q]) * recip[q] ----
        # The key-sum reduction (s_ps) and reciprocal are emitted after
        # eb=0's matmul group: their inputs are ready at phase-B end, so
        # the PE never stalls on the exp->accumulate tail, and the recip
        # lands on the DVE before eb=0's normalization multiplies need it.
        for eb in range(EB):
            psq = [p_ps.tile([P, CH], F32, tag="ps", name="ps") for _ in range(QC)]
            for kb in range(KB):
                for qc in range(QC):
                    nc.tensor.matmul(
                        psq[qc][:],
                        v_sb[:, kb, eb * P : (eb + 1) * P],
                        es_sb[:, kb, qc * CH : (qc + 1) * CH],
                        start=(kb == 0),
                        stop=(kb == KB - 1),
                    )
            if eb == 0:
                for qc in range(QC):
                    sp = p_pss.tile([P, CH], F32, tag="sps", name="s_ps")
                    nc.tensor.matmul(
                        sp[:], ones32_sb[:], acc_sb[:, qc, :], start=True, stop=True
                    )
                    # bf16 recip: ~0.2% uniform scale noise per query —
                    # well inside the error budget, saves 2KB/partition.
                    with nc.allow_low_precision(reason="bf16 softmax recip"):
                        nc.vector.reciprocal(
                            recip_sb[:, qc * CH : (qc + 1) * CH], sp[:]
                        )
            for qc in range(QC):
                nc.vector.tensor_mul(
                    vtu_sb[:, eb, qc * CH : (qc + 1) * CH],
                    psq[qc][:],
                    recip_sb[:, qc * CH : (qc + 1) * CH],
                )

        p_o = tc.alloc_tile_pool(name="o", bufs=3, side="left")

        # ---- Phase D: O[q, f] = V_.T' @ Wo.T + bo ----
        for qb in range(QB):
            ot = p_o.tile([P, D], F32, tag="ot", name="ot")
            for fc in range(FC):
                ps = p_ps.tile([P, CH], F32, tag="ps", name="ps")
                for eb in range(EB):
                    nc.tensor.matmul(
                        ps[:],
                        vtu_sb[:, eb, qb * P : (qb + 1) * P],
                        wot_sb[:, eb, fc * CH : (fc + 1) * CH],
                        start=(eb == 0),
                        stop=(eb == EB - 1),
                    )
                nc.vector.tensor_add(
                    ot[:, fc * CH : (fc + 1) * CH],
                    ps[:],
                    bob_sb[:, fc * CH : (fc + 1) * CH],
                )
            # per-chunk stores so the first half ships while the second
            # half's add is still running
            for fc in range(FC):
                dma_act(
                    out=o_d[qb * P : (qb + 1) * P, fc * CH : (fc + 1) * CH],
                    in_=ot[:, fc * CH : (fc + 1) * CH],
                )

        p_es.release()
        p_tmp.release()
        p_cs.release()
        p_vs.release()
        p_xs.release()
        p_v.release()
        p_o.release()
        p_vtu.release()
        p_wa.release()
        p_misc.release()
        p_dram.release()
        p_pss.release()
        p_ps.release()

    nc.finalize()
    return nc


def get_nc() -> bass.Bass:
    global _NC_CACHE
    if _NC_CACHE is None:
        _NC_CACHE = _build_nc()
    return _NC_CACHE


def make_in_maps(inputs: dict) -> list[dict]:
    _K = np.asarray(inputs["_K"], dtype=np.float32)
    _V = np.asarray(inputs["_V"], dtype=np.float32)
    _Q = np.asarray(inputs["_Q"], dtype=np.float32)

    shared = {
        "wkt": np.ascontiguousarray(
            np.asarray(inputs["Wk"], np.float32).T.astype(NPBF16)
        ),
        "wqt": np.ascontiguousarray(
            np.asarray(inputs["Wq"], np.float32).T.astype(NPBF16)
        ),
        "wvt": np.ascontiguousarray(
            np.asarray(inputs["Wv"], np.float32).T.astype(NPBF16)
        ),
        "wot": np.ascontiguousarray(
            np.asarray(inputs["Wo"], np.float32).T.astype(NPBF16)
        ),
        "bk": np.ascontiguousarray(
            np.asarray(inputs["bk"], np.float32).reshape(EB, P).T
        ),
        "bq": np.ascontiguousarray(
            np.asarray(inputs["bq"], np.float32).reshape(EB, P).T
        ),
        "bvb": np.ascontiguousarray(
            np.broadcast_to(
                np.asarray(inputs["bv"], np.float32).astype(NPBF16), (P, D)
            )
        ),
        "bob": np.ascontiguousarray(
            np.broadcast_to(
                np.asarray(inputs["bo"], np.float32).astype(NPBF16), (P, D)
            )
        ),
    }

    in_maps = []
    for c in range(8):
        b, h = divmod(c, 2)
        # Each core projects its own key/value half; the pair AllGather +
        # on-chip recovery fills the partner half. Local key order is
        # [own half; partner half] — valid because softmax and the
        # V-weighted sum are key-order invariant.
        kt = np.ascontiguousarray(
            _K[b, h * SH : (h + 1) * SH, :].T.astype(NPBF16)
        )
        vt = np.ascontiguousarray(
            _V[b, h * SH : (h + 1) * SH, :].T.astype(NPBF16)
        )
        qt = np.ascontiguousarray(
            _Q[b, h * SQ : (h + 1) * SQ, :].T.astype(NPBF16)
        )
        in_maps.append({"kt": kt, "vt": vt, "qt": qt, **shared})
    return in_maps


def kernel(**inputs) -> np.ndarray:
    global LAST_EXEC_NS
    nc = get_nc()
    in_maps = make_in_maps(inputs)
    kwargs = {}
    if TRACE and TRACE_ALL_CORES:
        kwargs["trace_cores"] = list(range(8))
    res = run_bass_kernel_spmd(
        nc, in_maps, core_ids=list(range(8)), trace=TRACE, **kwargs
    )
    LAST_EXEC_NS = res.exec_time_ns

    out = np.empty((B, S, D), dtype=np.float32)
    for c in range(8):
        b, h = divmod(c, 2)
        out[b, h * SQ : (h + 1) * SQ, :] = res.results[c]["o"]
    return out


# revision 13
# speedup vs baseline: 1.1896x; 1.1896x over previous
"""Trainium2 Bass kernel for single-head attention (nn_MultiHeadAttention).

Reference computation (B=4, S=2048, D=1024, fp32):
    K = _K @ Wk.T + bk ; V = _V @ Wv.T + bv ; Q = _Q @ Wq.T + bq
    scores[b,k,q] = (K[b,k,:] . Q[b,q,:]) / sqrt(D)
    alpha = softmax(scores, axis=keys)
    V_[b,q,:] = sum_k V[b,k,:] * alpha[b,k,q]
    O = V_ @ Wo.T + bo

Sharding: core c = (b, h) with b = c//2 (batch), h = c%2 (query half of
1024). Each core handles the full key/value sequence of its batch and a
1024-query slice. Cores {2b, 2b+1} share batch b: each projects half the
keys/values and the halves are exchanged pair-wise with an AllGather.

Structural choices (all aimed at zero PE idle — the HAM clock gate
re-throttles the array from 2.4 to 1.2 GHz after ~3.4us idle):
  - Own-half-in-place exchange: each core's projected half goes straight
    to its final SBUF slot (local key order = [own half; partner half];
    softmax and the V-weighted sum are key-order invariant so any local
    order is valid). The pair AllGather output holds [rank0; rank1]; the
    partner half is recovered rank-agnostically as
    (r0 - own) + r1 with an fp32 intermediate — exact, because one of
    r0/r1 is bit-identical to own. Phase B scores own-half keys while
    the collective is still in flight.
  - All input streams are loaded as [128, 1024] tiles (2KB per partition
    line — small descriptors measurably halve effective DMA bandwidth)
    and are fully resident before the collective window opens, so the
    collective's HBM traffic never contends with the input streams.
  - DMA queue split: input streams + gather-back loads on the Sync DGE
    queue; collective staging stores + output stores on the Activation
    DGE queue.
  - A burst of dummy matmuls fed from the first weight block (no memset
    dependency — the Vector engine's start preamble is ~7us) warms the
    HAM clock gate before real work; a tiny AllGather issued first
    absorbs the one-time comm-init barrier.
  - Key-sums for softmax: exp tiles accumulate on GpSimd (otherwise
    idle), reduced across partitions at the end with one fp32
    ones-matmul per query chunk instead of 32 per-block ones-matmuls.
All main matmuls are bf16 (M=128, N=512) accumulating in fp32 PSUM.
"""

import sys

if "/opt/trn_rl_repo" not in sys.path:
    sys.path.insert(0, "/opt/trn_rl_repo")

import ml_dtypes
import numpy as np

import concourse.bass as bass
import concourse.tile as tile
from concourse import bacc, mybir
from concourse.bass_utils import run_bass_kernel_spmd

B, S, D = 4, 2048, 1024
SQ = 1024  # queries per core
SH = 1024  # keys/values projected per core (pair exchange fills the rest)
P = 128  # partitions
CH = 512  # matmul moving free dim (one fp32 PSUM bank)
EB = D // P  # 8 feature blocks
DB = D // P  # 8 contraction blocks
KB = S // P  # 16 key blocks
QB = SQ // P  # 8 query blocks
QC = SQ // CH  # 2 query chunks
FC = D // CH  # 2 output-feature chunks
SCALE = 1.0 / np.sqrt(np.float32(D))  # folded into exp()

F32 = mybir.dt.float32
BF16 = mybir.dt.bfloat16
AF = mybir.ActivationFunctionType
NPBF16 = ml_dtypes.bfloat16

N_WARM_MM = 16  # dummy matmuls to flip the HAM clock gate before real work

# test.py can flip this to get a profiled run; the measured NEFF time (max
# over traced cores) lands in LAST_EXEC_NS.
TRACE = False
TRACE_ALL_CORES = False
LAST_EXEC_NS = None

_NC_CACHE = None


def _build_nc() -> bass.Bass:
    # Bacc (not plain Bass): its finalize() pipeline splits multi-sem waits
    # into event-semaphore chains — TRN2 instructions take at most 1 wait.
    nc = bacc.Bacc(num_devices=8)

    kt_d = nc.dram_tensor("kt", [D, SH], BF16, kind="ExternalInput")
    vt_d = nc.dram_tensor("vt", [D, SH], BF16, kind="ExternalInput")
    qt_d = nc.dram_tensor("qt", [D, SQ], BF16, kind="ExternalInput")
    wkt_d = nc.dram_tensor("wkt", [D, D], BF16, kind="ExternalInput")
    wqt_d = nc.dram_tensor("wqt", [D, D], BF16, kind="ExternalInput")
    wvt_d = nc.dram_tensor("wvt", [D, D], BF16, kind="ExternalInput")
    wot_d = nc.dram_tensor("wot", [D, D], BF16, kind="ExternalInput")
    bk_d = nc.dram_tensor("bk", [P, EB], F32, kind="ExternalInput")
    bq_d = nc.dram_tensor("bq", [P, EB], F32, kind="ExternalInput")
    bvb_d = nc.dram_tensor("bvb", [P, D], BF16, kind="ExternalInput")
    bob_d = nc.dram_tensor("bob", [P, D], BF16, kind="ExternalInput")
    o_d = nc.dram_tensor("o", [SQ, D], F32, kind="ExternalOutput")

    with tile.TileContext(nc) as tc:
        # Pools are stack-allocated per SBUF side. The weight ring (wa,
        # bufs=2) cycles wkt->wqt->wvt->wot through two 16KB slots; each
        # reuse WAR-waits only on the previous phase's matmuls, which are
        # done long before the next weight is needed. kt/qt (kqt) are
        # ACT/DVE-written only, so that region is safely recycled for vtu
        # once phase B is done.
        p_misc = tc.alloc_tile_pool(name="misc", bufs=1, side="left")
        p_ps = tc.alloc_tile_pool(name="ps", bufs=6, space="PSUM")
        p_pss = tc.alloc_tile_pool(name="pss", bufs=2, space="PSUM")
        p_v = tc.alloc_tile_pool(name="v", bufs=1, side="right")
        p_xs = tc.alloc_tile_pool(name="xs", bufs=12, side="right")
        p_vs = tc.alloc_tile_pool(name="vs", bufs=8, side="right")
        p_cs = tc.alloc_tile_pool(name="cs", bufs=2, side="right")
        p_tmp = tc.alloc_tile_pool(name="tmp", bufs=1, side="right")
        p_wa = tc.alloc_tile_pool(name="wa", bufs=2, side="left")
        p_kqt = tc.alloc_tile_pool(name="kqt", bufs=1, side="left")

        p_dram = tc.alloc_tile_pool(name="dram", bufs=1, space="DRAM")

        dma = nc.sync.dma_start
        dma_act = nc.scalar.dma_start

        recip_sb = p_misc.tile([P, SQ], F32)
        ones32_sb = p_misc.tile([P, P], F32)  # key-sum partition reduction
        acc_sb = p_misc.tile([P, QC, CH], F32)  # exp-sum accumulators
        nc.vector.memset(ones32_sb[:], 1.0)

        # Pair-wise exchange groups: {2b, 2b+1} share batch b.
        # The first collective pays a large one-time comm-init cost, so a
        # 128-byte warmup AllGather is issued immediately (staged via the
        # fast Sync DGE — the gpsimd software DGE takes ~10us to build
        # descriptors) and initializes the channels during phase A.
        CC_GROUPS = [[0, 1], [2, 3], [4, 5], [6, 7]]
        warm_in = p_dram.tile([1, 64], BF16)
        warm_out = p_dram.tile([2, 64], BF16)
        dma(out=warm_in[:], in_=kt_d[0:1, 0:64])
        nc.gpsimd.collective_compute(
            "AllGather",
            mybir.AluOpType.bypass,
            replica_groups=CC_GROUPS,
            ins=[warm_in.opt()],
            outs=[warm_out.opt()],
        )
        cc_kin = p_dram.tile([D, SH], BF16)
        cc_kout = p_dram.tile([2 * D, SH], BF16)
        cc_vin = p_dram.tile([SH, D], BF16)
        cc_vout = p_dram.tile([2 * SH, D], BF16)

        # One DMA per d-block so loads spread across HW queues and each
        # matmul depends only on its own 256KB slice.
        def load_w(name):
            t = p_wa.tile([P, DB, D], BF16, tag="w", name=name)
            src = {"wkt": wkt_d, "wqt": wqt_d, "wvt": wvt_d, "wot": wot_d}[
                name
            ].rearrange("(a p) e -> p a e", p=P)
            for a in range(DB):
                dma(out=t[:, a, :], in_=src[:, a, :])
            return t

        wkt_sb = load_w("wkt")
        bk_sb = p_misc.tile([P, EB], F32)
        dma(out=bk_sb[:], in_=bk_d[:])
        bq_sb = p_misc.tile([P, EB], F32)
        dma(out=bq_sb[:], in_=bq_d[:])

        # ---- PE warmup: flip the HAM clock gate to 2.4 GHz while the
        # input DMAs are still in flight. The dummy matmuls read the first
        # weight block (lands ~3us — a memset operand would be gated on
        # the Vector engine's ~7us start preamble) and write a scratch
        # PSUM slot recycled by the first projection group.
        warm_ps = p_ps.tile([P, CH], F32, tag="ps", name="ps")
        for _ in range(N_WARM_MM):
            nc.tensor.matmul(
                warm_ps[:, 0:P],
                wkt_sb[:, 0, 0:P],
                wkt_sb[:, 0, 0:P],
                start=True,
                stop=True,
            )

        kt_sb = p_kqt.tile([P, EB, S], BF16)  # K.T: [e_p, e_blk, k]
        qt_sb = p_kqt.tile([P, EB, SQ], BF16)  # Q.T: [e_p, e_blk, q]
        v_sb = p_v.tile([P, KB, D], BF16)  # V:   [k_p, k_blk, e]

        # ---- Phase A: projections ----
        # Q.T and K.T: out[e, s] = sum_d W.T[d, e] (stationary) @ _X.T[d, s]
        # Input streams are [128, 1024] tiles; the matmuls slice the moving
        # operand per 512-chunk. The K stream rides the Activation DGE
        # queue so it lands in parallel with the weights on the Sync queue.
        def kq_proj(proj_w, proj_in, proj_out, proj_b, nchunk, dma_x=dma):
            xtt = []
            for d_ in range(DB):
                t = p_xs.tile([P, SH], BF16, tag="xtt", name="xtt")
                dma_x(out=t[:], in_=proj_in[d_ * P : (d_ + 1) * P, :])
                xtt.append(t)
            for sc in range(nchunk):
                for eb in range(EB):
                    ps = p_ps.tile([P, CH], F32, tag="ps", name="ps")
                    for d_ in range(DB):
                        nc.tensor.matmul(
                            ps[:],
                            proj_w[:, d_, eb * P : (eb + 1) * P],
                            xtt[d_][:, sc * CH : (sc + 1) * CH],
                            start=(d_ == 0),
                            stop=(d_ == DB - 1),
                        )
                    # DVE, not ACT: ~3x faster per copy-out, frees the psum
                    # slot sooner, and keeps ScalarE clear for phase B's exp
                    nc.vector.tensor_scalar_add(
                        proj_out[:, eb, sc * CH : (sc + 1) * CH],
                        ps[:],
                        proj_b[:, eb : eb + 1],
                    )

        # K.T own half lands directly in its final slot kt_sb[:, :, 0:SH].
        kq_proj(wkt_sb, kt_d, kt_sb, bk_sb, SH // CH, dma_x=dma_act)
        for eb in range(EB):
            dma_act(out=cc_kin[eb * P : (eb + 1) * P, :], in_=kt_sb[:, eb, 0:SH])
        nc.gpsimd.collective_compute(
            "AllGather",
            mybir.AluOpType.bypass,
            replica_groups=CC_GROUPS,
            ins=[cc_kin.opt()],
            outs=[cc_kout.opt()],
        )

        wqt_sb = load_w("wqt")
        kq_proj(wqt_sb, qt_d, qt_sb, bq_sb, QC)

        wvt_sb = load_w("wvt")
        bvb_sb = p_misc.tile([P, D], BF16)
        dma(out=bvb_sb[:], in_=bvb_d[:])

        # V natural: out[k, e] = sum_d _V.T[d, k] (stationary) @ Wv.T[d, e]
        vtb = []
        for d_ in range(DB):
            t = p_vs.tile([P, SH], BF16, tag="vtt", name="vtt")
            dma(out=t[:], in_=vt_d[d_ * P : (d_ + 1) * P, :])
            vtb.append(t)
        for kb in range(SH // P):
            pse = [p_ps.tile([P, CH], F32, tag="ps", name="ps") for _ in range(FC)]
            for d_ in range(DB):
                for eh in range(FC):
                    nc.tensor.matmul(
                        pse[eh][:],
                        vtb[d_][:, kb * P : (kb + 1) * P],
                        wvt_sb[:, d_, eh * CH : (eh + 1) * CH],
                        start=(d_ == 0),
                        stop=(d_ == DB - 1),
                    )
            for eh in range(FC):
                nc.vector.tensor_add(
                    v_sb[:, kb, eh * CH : (eh + 1) * CH],
                    pse[eh][:],
                    bvb_sb[:, eh * CH : (eh + 1) * CH],
                )

        # V pair exchange (own half already in v_sb[:, 0:8, :])
        for kb in range(SH // P):
            dma_act(out=cc_vin[kb * P : (kb + 1) * P, :], in_=v_sb[:, kb, :])
        nc.gpsimd.collective_compute(
            "AllGather",
            mybir.AluOpType.bypass,
            replica_groups=CC_GROUPS,
            ins=[cc_vin.opt()],
            outs=[cc_vout.opt()],
        )

        wot_sb = load_w("wot")
        bob_sb = p_misc.tile([P, D], BF16)
        dma(out=bob_sb[:], in_=bob_d[:])

        # Partner K half: kt_sb[:, eb, SH:] = (r0 - own) + r1. One of
        # r0/r1 is bit-identical to own, so with the fp32 intermediate the
        # recovery is exact on both ranks.
        for eb in range(EB):
            la = p_cs.tile([P, SH], BF16, tag="cs", name="cs")
            dma(out=la[:], in_=cc_kout[eb * P : (eb + 1) * P, :])
            lb = p_cs.tile([P, SH], BF16, tag="cs", name="cs")
            dma(out=lb[:], in_=cc_kout[D + eb * P : D + (eb + 1) * P, :])
            t32 = p_tmp.tile([P, SH], F32, tag="t32", name="t32")
            nc.vector.tensor_sub(t32[:], la[:], kt_sb[:, eb, 0:SH])
            nc.vector.tensor_add(kt_sb[:, eb, SH:S], t32[:], lb[:])

        # Partner V half: v_sb[:, 8+kb, :] = (r0 - own) + r1.
        for kb in range(SH // P):
            la = p_cs.tile([P, D], BF16, tag="cs", name="cs")
            dma(out=la[:], in_=cc_vout[kb * P : (kb + 1) * P, :])
            lb = p_cs.tile([P, D], BF16, tag="cs", name="cs")
            dma(out=lb[:], in_=cc_vout[SH + kb * P : SH + (kb + 1) * P, :])
            t32 = p_tmp.tile([P, D], F32, tag="t32", name="t32")
            nc.vector.tensor_sub(t32[:], la[:], v_sb[:, kb, :])
            nc.vector.tensor_add(v_sb[:, 8 + kb, :], t32[:], lb[:])

        p_es = tc.alloc_tile_pool(name="es", bufs=1, side="right")
        es_sb = p_es.tile([P, KB, SQ], BF16)  # exp(scores): [k_p, k_blk, q]

        # ---- Phase B: scores[k, q] = K.T' @ Q.T, exp, and key-sums ----
        # Key blocks 0..7 are the own half (ready right after the K
        # projection); 8..15 are the partner half (recovered well before
        # the PE's in-order queue reaches them). exp tiles accumulate on
        # GpSimd; the partition reduction happens once at the end with a
        # single fp32 ones-matmul per query chunk.
        nc.gpsimd.memset(acc_sb[:], 0.0)
        for kb in range(KB):
            psq = [p_ps.tile([P, CH], F32, tag="ps", name="ps") for _ in range(QC)]
            for eb in range(EB):
                for qc in range(QC):
                    nc.tensor.matmul(
                        psq[qc][:],
                        kt_sb[:, eb, kb * P : (kb + 1) * P],
                        qt_sb[:, eb, qc * CH : (qc + 1) * CH],
                        start=(eb == 0),
                        stop=(eb == EB - 1),
                    )
            for qc in range(QC):
                nc.scalar.activation(
                    es_sb[:, kb, qc * CH : (qc + 1) * CH],
                    psq[qc][:],
                    AF.Exp,
                    scale=float(SCALE),
                )
                nc.gpsimd.tensor_add(
                    acc_sb[:, qc, :],
                    acc_sb[:, qc, :],
                    es_sb[:, kb, qc * CH : (qc + 1) * CH],
                )

        p_kqt.release()
        p_vtu = tc.alloc_tile_pool(name="vtu", bufs=1, side="left")
        vtu_sb = p_vtu.tile([P, EB, SQ], BF16)  # normalized V_.T: [e_p, e_blk, q]

        # ---- Phase C: V_.T[e, q] = (sum_k V[k, e] es[k, q]) * recip[q] ----
        # The key-sum reduction (s_ps) and reciprocal are emitted after
        # eb=0's matmul group: their inputs are ready at phase-B end, so
        # the PE never stalls on the exp->accumulate tail, and the recip
        # lands on the DVE before eb=0's normalization multiplies need it.
        for eb in range(EB):
            psq = [p_ps.tile([P, CH], F32, tag="ps", name="ps") for _ in range(QC)]
            for kb in range(KB):
                for qc in range(QC):
                    nc.tensor.matmul(
                        psq[qc][:],
                        v_sb[:, kb, eb * P : (eb + 1) * P],
                        es_sb[:, kb, qc * CH : (qc + 1) * CH],
                        start=(kb == 0),
                        stop=(kb == KB - 1),
                    )
            if eb == 0:
                for qc in range(QC):
                    sp = p_pss.tile([P, CH], F32, tag="sps", name="s_ps")
                    nc.tensor.matmul(
                        sp[:], ones32_sb[:], acc_sb[:, qc, :], start=True, stop=True
                    )
                    nc.vector.reciprocal(recip_sb[:, qc * CH : (qc + 1) * CH], sp[:])
            for qc in range(QC):
                nc.vector.tensor_mul(
                    vtu_sb[:, eb, qc * CH : (qc + 1) * CH],
                    psq[qc][:],
                    recip_sb[:, qc * CH : (qc + 1) * CH],
                )

        p_o = tc.alloc_tile_pool(name="o", bufs=3, side="left")

        # ---- Phase D: O[q, f] = V_.T' @ Wo.T + bo ----
        for qb in range(QB):
            ot = p_o.tile([P, D], F32, tag="ot", name="ot")
            for fc in range(FC):
                ps = p_ps.tile([P, CH], F32, tag="ps", name="ps")
                for eb in range(EB):
                    nc.tensor.matmul(
                        ps[:],
                        vtu_sb[:, eb, qb * P : (qb + 1) * P],
                        wot_sb[:, eb, fc * CH : (fc + 1) * CH],
                        start=(eb == 0),
                        stop=(eb == EB - 1),
                    )
                nc.vector.tensor_add(
                    ot[:, fc * CH : (fc + 1) * CH],
                    ps[:],
                    bob_sb[:, fc * CH : (fc + 1) * CH],
                )
            # per-chunk stores so the first half ships while the second
            # half's add is still running
            for fc in range(FC):
                dma_act(
                    out=o_d[qb * P : (qb + 1) * P, fc * CH : (fc + 1) * CH],
                    in_=ot[:, fc * CH : (fc + 1) * CH],
                )

        p_es.release()
        p_tmp.release()
        p_cs.release()
        p_vs.release()
        p_xs.release()
        p_v.release()
        p_o.release()
        p_vtu.release()
        p_wa.release()
        p_misc.release()
        p_dram.release()
        p_pss.release()
        p_ps.release()

    nc.finalize()
    return nc


def get_nc() -> bass.Bass:
    global _NC_CACHE
    if _NC_CACHE is None:
        _NC_CACHE = _build_nc()
    return _NC_CACHE


def make_in_maps(inputs: dict) -> list[dict]:
    _K = np.asarray(inputs["_K"], dtype=np.float32)
    _V = np.asarray(inputs["_V"], dtype=np.float32)
    _Q = np.asarray(inputs["_Q"], dtype=np.float32)

    shared = {
        "wkt": np.ascontiguousarray(
            np.asarray(inputs["Wk"], np.float32).T.astype(NPBF16)
        ),
        "wqt": np.ascontiguousarray(
            np.asarray(inputs["Wq"], np.float32).T.astype(NPBF16)
        ),
        "wvt": np.ascontiguousarray(
            np.asarray(inputs["Wv"], np.float32).T.astype(NPBF16)
        ),
        "wot": np.ascontiguousarray(
            np.asarray(inputs["Wo"], np.float32).T.astype(NPBF16)
        ),
        "bk": np.ascontiguousarray(
            np.asarray(inputs["bk"], np.float32).reshape(EB, P).T
        ),
        "bq": np.ascontiguousarray(
            np.asarray(inputs["bq"], np.float32).reshape(EB, P).T
        ),
        "bvb": np.ascontiguousarray(
            np.broadcast_to(
                np.asarray(inputs["bv"], np.float32).astype(NPBF16), (P, D)
            )
        ),
        "bob": np.ascontiguousarray(
            np.broadcast_to(
                np.asarray(inputs["bo"], np.float32).astype(NPBF16), (P, D)
            )
        ),
    }

    in_maps = []
    for c in range(8):
        b, h = divmod(c, 2)
        # Each core projects its own key/value half; the pair AllGather +
        # on-chip recovery fills the partner half. Local key order is
        # [own half; partner half] — valid because softmax and the
        # V-weighted sum are key-order invariant.
        kt = np.ascontiguousarray(
            _K[b, h * SH : (h + 1) * SH, :].T.astype(NPBF16)
        )
        vt = np.ascontiguousarray(
            _V[b, h * SH : (h + 1) * SH, :].T.astype(NPBF16)
        )
        qt = np.ascontiguousarray(
            _Q[b, h * SQ : (h + 1) * SQ, :].T.astype(NPBF16)
        )
        in_maps.append({"kt": kt, "vt": vt, "qt": qt, **shared})
    return in_maps


def kernel(**inputs) -> np.ndarray:
    global LAST_EXEC_NS
    nc = get_nc()
    in_maps = make_in_maps(inputs)
    kwargs = {}
    if TRACE and TRACE_ALL_CORES:
        kwargs["trace_cores"] = list(range(8))
    res = run_bass_kernel_spmd(
        nc, in_maps, core_ids=list(range(8)), trace=TRACE, **kwargs
    )
    LAST_EXEC_NS = res.exec_time_ns

    out = np.empty((B, S, D), dtype=np.float32)
    for c in range(8):
        b, h = divmod(c, 2)
        out[b, h * SQ : (h + 1) * SQ, :] = res.results[c]["o"]
    return out


# revision 14
# speedup vs baseline: 1.1901x; 1.0004x over previous
"""Trainium2 Bass kernel for single-head attention (nn_MultiHeadAttention).

Reference computation (B=4, S=2048, D=1024, fp32):
    K = _K @ Wk.T + bk ; V = _V @ Wv.T + bv ; Q = _Q @ Wq.T + bq
    scores[b,k,q] = (K[b,k,:] . Q[b,q,:]) / sqrt(D)
    alpha = softmax(scores, axis=keys)
    V_[b,q,:] = sum_k V[b,k,:] * alpha[b,k,q]
    O = V_ @ Wo.T + bo

Sharding: core c = (b, h) with b = c//2 (batch), h = c%2 (query half of
1024). Each core handles the full key/value sequence of its batch and a
1024-query slice. Cores {2b, 2b+1} share batch b: each projects half the
keys/values and the halves are exchanged pair-wise with an AllGather.

Structural choices (all aimed at zero PE idle — the HAM clock gate
re-throttles the array from 2.4 to 1.2 GHz after ~3.4us idle):
  - Own-half-in-place exchange: each core's projected half goes straight
    to its final SBUF slot (local key order = [own half; partner half];
    softmax and the V-weighted sum are key-order invariant so any local
    order is valid). The pair AllGather output holds [rank0; rank1]; the
    partner half is recovered rank-agnostically as
    (r0 - own) + r1 with an fp32 intermediate — exact, because one of
    r0/r1 is bit-identical to own. Phase B scores own-half keys while
    the collective is still in flight.
  - All input streams are loaded as [128, 1024] tiles (2KB per partition
    line — small descriptors measurably halve effective DMA bandwidth)
    and are fully resident before the collective window opens, so the
    collective's HBM traffic never contends with the input streams.
  - DMA queue split: input streams + gather-back loads on the Sync DGE
    queue; collective staging stores + output stores on the Activation
    DGE queue.
  - A burst of dummy matmuls fed from the first weight block (no memset
    dependency — the Vector engine's start preamble is ~7us) warms the
    HAM clock gate before real work; a tiny AllGather issued first
    absorbs the one-time comm-init barrier.
  - Key-sums for softmax: exp tiles accumulate on GpSimd (otherwise
    idle), reduced across partitions at the end with one fp32
    ones-matmul per query chunk instead of 32 per-block ones-matmuls.
All main matmuls are bf16 (M=128, N=512) accumulating in fp32 PSUM.
"""

import sys

if "/opt/trn_rl_repo" not in sys.path:
    sys.path.insert(0, "/opt/trn_rl_repo")

import ml_dtypes
import numpy as np

import concourse.bass as bass
import concourse.tile as tile
from concourse import bacc, mybir
from concourse.bass_utils import run_bass_kernel_spmd

B, S, D = 4, 2048, 1024
SQ = 1024  # queries per core
SH = 1024  # keys/values projected per core (pair exchange fills the rest)
P = 128  # partitions
CH = 512  # matmul moving free dim (one fp32 PSUM bank)
EB = D // P  # 8 feature blocks
DB = D // P  # 8 contraction blocks
KB = S // P  # 16 key blocks
QB = SQ // P  # 8 query blocks
QC = SQ // CH  # 2 query chunks
FC = D // CH  # 2 output-feature chunks
SCALE = 1.0 / np.sqrt(np.float32(D))  # folded into exp()

F32 = mybir.dt.float32
BF16 = mybir.dt.bfloat16
AF = mybir.ActivationFunctionType
NPBF16 = ml_dtypes.bfloat16

N_WARM_MM = 16  # dummy matmuls to flip the HAM clock gate before real work

# test.py can flip this to get a profiled run; the measured NEFF time (max
# over traced cores) lands in LAST_EXEC_NS.
TRACE = False
TRACE_ALL_CORES = False
LAST_EXEC_NS = None

_NC_CACHE = None


def _build_nc() -> bass.Bass:
    # Bacc (not plain Bass): its finalize() pipeline splits multi-sem waits
    # into event-semaphore chains — TRN2 instructions take at most 1 wait.
    nc = bacc.Bacc(num_devices=8)

    kt_d = nc.dram_tensor("kt", [D, SH], BF16, kind="ExternalInput")
    vt_d = nc.dram_tensor("vt", [D, SH], BF16, kind="ExternalInput")
    qt_d = nc.dram_tensor("qt", [D, SQ], BF16, kind="ExternalInput")
    wkt_d = nc.dram_tensor("wkt", [D, D], BF16, kind="ExternalInput")
    wqt_d = nc.dram_tensor("wqt", [D, D], BF16, kind="ExternalInput")
    wvt_d = nc.dram_tensor("wvt", [D, D], BF16, kind="ExternalInput")
    wot_d = nc.dram_tensor("wot", [D, D], BF16, kind="ExternalInput")
    bk_d = nc.dram_tensor("bk", [P, EB], F32, kind="ExternalInput")
    bq_d = nc.dram_tensor("bq", [P, EB], F32, kind="ExternalInput")
    bvb_d = nc.dram_tensor("bvb", [P, D], BF16, kind="ExternalInput")
    bob_d = nc.dram_tensor("bob", [P, D], BF16, kind="ExternalInput")
    o_d = nc.dram_tensor("o", [SQ, D], F32, kind="ExternalOutput")

    with tile.TileContext(nc) as tc:
        # Pools are stack-allocated per SBUF side. The weight ring (wa,
        # bufs=2) cycles wkt->wqt->wvt->wot through two 16KB slots; each
        # reuse WAR-waits only on the previous phase's matmuls, which are
        # done long before the next weight is needed. kt/qt (kqt) are
        # ACT/DVE-written only, so that region is safely recycled for vtu
        # once phase B is done.
        p_misc = tc.alloc_tile_pool(name="misc", bufs=1, side="left")
        p_ps = tc.alloc_tile_pool(name="ps", bufs=6, space="PSUM")
        p_pss = tc.alloc_tile_pool(name="pss", bufs=2, space="PSUM")
        p_v = tc.alloc_tile_pool(name="v", bufs=1, side="right")
        p_xs = tc.alloc_tile_pool(name="xs", bufs=8, side="right")
        p_vs = tc.alloc_tile_pool(name="vs", bufs=8, side="right")
        p_cs = tc.alloc_tile_pool(name="cs", bufs=4, side="right")
        p_tmp = tc.alloc_tile_pool(name="tmp", bufs=2, side="right")
        p_wa = tc.alloc_tile_pool(name="wa", bufs=2, side="left")
        p_kqt = tc.alloc_tile_pool(name="kqt", bufs=1, side="left")

        p_dram = tc.alloc_tile_pool(name="dram", bufs=1, space="DRAM")

        dma = nc.sync.dma_start
        dma_act = nc.scalar.dma_start

        recip_sb = p_misc.tile([P, SQ], BF16)
        ones32_sb = p_misc.tile([P, P], F32)  # key-sum partition reduction
        acc_sb = p_misc.tile([P, QC, CH], F32)  # exp-sum accumulators
        nc.vector.memset(ones32_sb[:], 1.0)

        # Pair-wise exchange groups: {2b, 2b+1} share batch b.
        # The first collective pays a large one-time comm-init cost, so a
        # 128-byte warmup AllGather is issued immediately (staged via the
        # fast Sync DGE — the gpsimd software DGE takes ~10us to build
        # descriptors) and initializes the channels during phase A.
        CC_GROUPS = [[0, 1], [2, 3], [4, 5], [6, 7]]
        warm_in = p_dram.tile([1, 64], BF16)
        warm_out = p_dram.tile([2, 64], BF16)
        dma(out=warm_in[:], in_=kt_d[0:1, 0:64])
        nc.gpsimd.collective_compute(
            "AllGather",
            mybir.AluOpType.bypass,
            replica_groups=CC_GROUPS,
            ins=[warm_in.opt()],
            outs=[warm_out.opt()],
        )
        cc_kin = p_dram.tile([D, SH], BF16)
        cc_kout = p_dram.tile([2 * D, SH], BF16)
        cc_vin = p_dram.tile([SH, D], BF16)
        cc_vout = p_dram.tile([2 * SH, D], BF16)

        # One DMA per d-block so loads spread across HW queues and each
        # matmul depends only on its own 256KB slice.
        def load_w(name):
            t = p_wa.tile([P, DB, D], BF16, tag="w", name=name)
            src = {"wkt": wkt_d, "wqt": wqt_d, "wvt": wvt_d, "wot": wot_d}[
                name
            ].rearrange("(a p) e -> p a e", p=P)
            for a in range(DB):
                dma(out=t[:, a, :], in_=src[:, a, :])
            return t

        wkt_sb = load_w("wkt")
        bk_sb = p_misc.tile([P, EB], F32)
        dma(out=bk_sb[:], in_=bk_d[:])
        bq_sb = p_misc.tile([P, EB], F32)
        dma(out=bq_sb[:], in_=bq_d[:])

        # ---- PE warmup: flip the HAM clock gate to 2.4 GHz while the
        # input DMAs are still in flight. The dummy matmuls read the first
        # weight block (lands ~3us — a memset operand would be gated on
        # the Vector engine's ~7us start preamble) and write a scratch
        # PSUM slot recycled by the first projection group.
        warm_ps = p_ps.tile([P, CH], F32, tag="ps", name="ps")
        for _ in range(N_WARM_MM):
            nc.tensor.matmul(
                warm_ps[:, 0:P],
                wkt_sb[:, 0, 0:P],
                wkt_sb[:, 0, 0:P],
                start=True,
                stop=True,
            )

        kt_sb = p_kqt.tile([P, EB, S], BF16)  # K.T: [e_p, e_blk, k]
        qt_sb = p_kqt.tile([P, EB, SQ], BF16)  # Q.T: [e_p, e_blk, q]
        v_sb = p_v.tile([P, KB, D], BF16)  # V:   [k_p, k_blk, e]

        # ---- Phase A: projections ----
        # Q.T and K.T: out[e, s] = sum_d W.T[d, e] (stationary) @ _X.T[d, s]
        # Input streams are [128, 1024] tiles; the matmuls slice the moving
        # operand per 512-chunk. The K stream rides the Activation DGE
        # queue so it lands in parallel with the weights on the Sync queue.
        def kq_proj(proj_w, proj_in, proj_out, proj_b, nchunk, dma_x=dma):
            xtt = []
            for d_ in range(DB):
                t = p_xs.tile([P, SH], BF16, tag="xtt", name="xtt")
                dma_x(out=t[:], in_=proj_in[d_ * P : (d_ + 1) * P, :])
                xtt.append(t)
            for sc in range(nchunk):
                for eb in range(EB):
                    ps = p_ps.tile([P, CH], F32, tag="ps", name="ps")
                    for d_ in range(DB):
                        nc.tensor.matmul(
                            ps[:],
                            proj_w[:, d_, eb * P : (eb + 1) * P],
                            xtt[d_][:, sc * CH : (sc + 1) * CH],
                            start=(d_ == 0),
                            stop=(d_ == DB - 1),
                        )
                    # DVE, not ACT: ~3x faster per copy-out, frees the psum
                    # slot sooner, and keeps ScalarE clear for phase B's exp
                    nc.vector.tensor_scalar_add(
                        proj_out[:, eb, sc * CH : (sc + 1) * CH],
                        ps[:],
                        proj_b[:, eb : eb + 1],
                    )

        # K.T own half lands directly in its final slot kt_sb[:, :, 0:SH].
        kq_proj(wkt_sb, kt_d, kt_sb, bk_sb, SH // CH, dma_x=dma_act)
        for eb in range(EB):
            dma_act(out=cc_kin[eb * P : (eb + 1) * P, :], in_=kt_sb[:, eb, 0:SH])
        nc.gpsimd.collective_compute(
            "AllGather",
            mybir.AluOpType.bypass,
            replica_groups=CC_GROUPS,
            ins=[cc_kin.opt()],
            outs=[cc_kout.opt()],
        )

        wvt_sb = load_w("wvt")
        bvb_sb = p_misc.tile([P, D], BF16)
        dma(out=bvb_sb[:], in_=bvb_d[:])

        # V natural: out[k, e] = sum_d _V.T[d, k] (stationary) @ Wv.T[d, e]
        vtb = []
        for d_ in range(DB):
            t = p_vs.tile([P, SH], BF16, tag="vtt", name="vtt")
            dma(out=t[:], in_=vt_d[d_ * P : (d_ + 1) * P, :])
            vtb.append(t)
        for kb in range(SH // P):
            pse = [p_ps.tile([P, CH], F32, tag="ps", name="ps") for _ in range(FC)]
            for d_ in range(DB):
                for eh in range(FC):
                    nc.tensor.matmul(
                        pse[eh][:],
                        vtb[d_][:, kb * P : (kb + 1) * P],
                        wvt_sb[:, d_, eh * CH : (eh + 1) * CH],
                        start=(d_ == 0),
                        stop=(d_ == DB - 1),
                    )
            for eh in range(FC):
                nc.vector.tensor_add(
                    v_sb[:, kb, eh * CH : (eh + 1) * CH],
                    pse[eh][:],
                    bvb_sb[:, eh * CH : (eh + 1) * CH],
                )

        # V pair exchange (own half already in v_sb[:, 0:8, :])
        for kb in range(SH // P):
            dma_act(out=cc_vin[kb * P : (kb + 1) * P, :], in_=v_sb[:, kb, :])
        nc.gpsimd.collective_compute(
            "AllGather",
            mybir.AluOpType.bypass,
            replica_groups=CC_GROUPS,
            ins=[cc_vin.opt()],
            outs=[cc_vout.opt()],
        )

        # Q projection last: its input tiles reuse the K stream's SBUF
        # slots (WAR on the K matmuls, which are long done), and phase B
        # follows it seamlessly on the PE.
        wqt_sb = load_w("wqt")
        kq_proj(wqt_sb, qt_d, qt_sb, bq_sb, QC)

        wot_sb = load_w("wot")
        bob_sb = p_misc.tile([P, D], BF16)
        dma(out=bob_sb[:], in_=bob_d[:])

        # Partner halves: partner = (r0 - own) + r1. One of r0/r1 is
        # bit-identical to own, so with the fp32 intermediate (512-wide
        # halves to keep the ring small) the recovery is exact on both
        # ranks.
        def recover(own_ap, out_ap, cc_out, row0):
            la = p_cs.tile([P, SH], BF16, tag="cs", name="cs")
            dma(out=la[:], in_=cc_out[row0 : row0 + P, :])
            lb = p_cs.tile([P, SH], BF16, tag="cs", name="cs")
            dma(out=lb[:], in_=cc_out[row0 + SH : row0 + SH + P, :])
            for hh in range(2):
                sl = slice(hh * CH, (hh + 1) * CH)
                t32 = p_tmp.tile([P, CH], F32, tag="t32", name="t32")
                nc.vector.tensor_sub(t32[:], la[:, sl], own_ap[:, sl])
                nc.vector.tensor_add(out_ap[:, sl], t32[:], lb[:, sl])

        for eb in range(EB):
            recover(kt_sb[:, eb, 0:SH], kt_sb[:, eb, SH:S], cc_kout, eb * P)
        for kb in range(SH // P):
            recover(v_sb[:, kb, :], v_sb[:, 8 + kb, :], cc_vout, kb * P)

        p_es = tc.alloc_tile_pool(name="es", bufs=1, side="right")
        es_sb = p_es.tile([P, KB, SQ], BF16)  # exp(scores): [k_p, k_blk, q]

        # ---- Phase B: scores[k, q] = K.T' @ Q.T, exp, and key-sums ----
        # Key blocks 0..7 are the own half (ready right after the K
        # projection); 8..15 are the partner half (recovered well before
        # the PE's in-order queue reaches them). exp tiles accumulate on
        # GpSimd; the partition reduction happens once at the end with a
        # single fp32 ones-matmul per query chunk.
        nc.gpsimd.memset(acc_sb[:], 0.0)
        for kb in range(KB):
            psq = [p_ps.tile([P, CH], F32, tag="ps", name="ps") for _ in range(QC)]
            for eb in range(EB):
                for qc in range(QC):
                    nc.tensor.matmul(
                        psq[qc][:],
                        kt_sb[:, eb, kb * P : (kb + 1) * P],
                        qt_sb[:, eb, qc * CH : (qc + 1) * CH],
                        start=(eb == 0),
                        stop=(eb == EB - 1),
                    )
            for qc in range(QC):
                nc.scalar.activation(
                    es_sb[:, kb, qc * CH : (qc + 1) * CH],
                    psq[qc][:],
                    AF.Exp,
                    scale=float(SCALE),
                )
                nc.gpsimd.tensor_add(
                    acc_sb[:, qc, :],
                    acc_sb[:, qc, :],
                    es_sb[:, kb, qc * CH : (qc + 1) * CH],
                )

        p_kqt.release()
        p_vtu = tc.alloc_tile_pool(name="vtu", bufs=1, side="left")
        vtu_sb = p_vtu.tile([P, EB, SQ], BF16)  # normalized V_.T: [e_p, e_blk, q]

        # ---- Phase C: V_.T[e, q] = (sum_k V[k, e] es[k, q]) * recip[q] ----
        # The key-sum reduction (s_ps) and reciprocal are emitted after
        # eb=0's matmul group: their inputs are ready at phase-B end, so
        # the PE never stalls on the exp->accumulate tail, and the recip
        # lands on the DVE before eb=0's normalization multiplies need it.
        for eb in range(EB):
            psq = [p_ps.tile([P, CH], F32, tag="ps", name="ps") for _ in range(QC)]
            for kb in range(KB):
                for qc in range(QC):
                    nc.tensor.matmul(
                        psq[qc][:],
                        v_sb[:, kb, eb * P : (eb + 1) * P],
                        es_sb[:, kb, qc * CH : (qc + 1) * CH],
                        start=(kb == 0),
                        stop=(kb == KB - 1),
                    )
            if eb == 0:
                for qc in range(QC):
                    sp = p_pss.tile([P, CH], F32, tag="sps", name="s_ps")
                    nc.tensor.matmul(
                        sp[:], ones32_sb[:], acc_sb[:, qc, :], start=True, stop=True
                    )
                    # bf16 recip: ~0.2% uniform scale noise per query —
                    # well inside the error budget, saves 2KB/partition.
                    with nc.allow_low_precision(reason="bf16 softmax recip"):
                        nc.vector.reciprocal(
                            recip_sb[:, qc * CH : (qc + 1) * CH], sp[:]
                        )
            for qc in range(QC):
                nc.vector.tensor_mul(
                    vtu_sb[:, eb, qc * CH : (qc + 1) * CH],
                    psq[qc][:],
                    recip_sb[:, qc * CH : (qc + 1) * CH],
                )

        p_o = tc.alloc_tile_pool(name="o", bufs=3, side="left")

        # ---- Phase D: O[q, f] = V_.T' @ Wo.T + bo ----
        for qb in range(QB):
            ot = p_o.tile([P, D], F32, tag="ot", name="ot")
            for fc in range(FC):
                ps = p_ps.tile([P, CH], F32, tag="ps", name="ps")
                for eb in range(EB):
                    nc.tensor.matmul(
                        ps[:],
                        vtu_sb[:, eb, qb * P : (qb + 1) * P],
                        wot_sb[:, eb, fc * CH : (fc + 1) * CH],
                        start=(eb == 0),
                        stop=(eb == EB - 1),
                    )
                nc.vector.tensor_add(
                    ot[:, fc * CH : (fc + 1) * CH],
                    ps[:],
                    bob_sb[:, fc * CH : (fc + 1) * CH],
                )
            # per-chunk stores so the first half ships while the second
            # half's add is still running
            for fc in range(FC):
                dma_act(
                    out=o_d[qb * P : (qb + 1) * P, fc * CH : (fc + 1) * CH],
                    in_=ot[:, fc * CH : (fc + 1) * CH],
                )

        p_es.release()
        p_tmp.release()
        p_cs.release()
        p_vs.release()
        p_xs.release()
        p_v.release()
        p_o.release()
        p_vtu.release()
        p_wa.release()
        p_misc.release()
        p_dram.release()
        p_pss.release()
        p_ps.release()

    nc.finalize()
    return nc


def get_nc() -> bass.Bass:
    global _NC_CACHE
    if _NC_CACHE is None:
        _NC_CACHE = _build_nc()
    return _NC_CACHE


def make_in_maps(inputs: dict) -> list[dict]:
    _K = np.asarray(inputs["_K"], dtype=np.float32)
    _V = np.asarray(inputs["_V"], dtype=np.float32)
    _Q = np.asarray(inputs["_Q"], dtype=np.float32)

    shared = {
        "wkt": np.ascontiguousarray(
            np.asarray(inputs["Wk"], np.float32).T.astype(NPBF16)
        ),
        "wqt": np.ascontiguousarray(
            np.asarray(inputs["Wq"], np.float32).T.astype(NPBF16)
        ),
        "wvt": np.ascontiguousarray(
            np.asarray(inputs["Wv"], np.float32).T.astype(NPBF16)
        ),
        "wot": np.ascontiguousarray(
            np.asarray(inputs["Wo"], np.float32).T.astype(NPBF16)
        ),
        "bk": np.ascontiguousarray(
            np.asarray(inputs["bk"], np.float32).reshape(EB, P).T
        ),
        "bq": np.ascontiguousarray(
            np.asarray(inputs["bq"], np.float32).reshape(EB, P).T
        ),
        "bvb": np.ascontiguousarray(
            np.broadcast_to(
                np.asarray(inputs["bv"], np.float32).astype(NPBF16), (P, D)
            )
        ),
        "bob": np.ascontiguousarray(
            np.broadcast_to(
                np.asarray(inputs["bo"], np.float32).astype(NPBF16), (P, D)
            )
        ),
    }

    in_maps = []
    for c in range(8):
        b, h = divmod(c, 2)
        # Each core projects its own key/value half; the pair AllGather +
        # on-chip recovery fills the partner half. Local key order is
        # [own half; partner half] — valid because softmax and the
        # V-weighted sum are key-order invariant.
        kt = np.ascontiguousarray(
            _K[b, h * SH : (h + 1) * SH, :].T.astype(NPBF16)
        )
        vt = np.ascontiguousarray(
            _V[b, h * SH : (h + 1) * SH, :].T.astype(NPBF16)
        )
        qt = np.ascontiguousarray(
            _Q[b, h * SQ : (h + 1) * SQ, :].T.astype(NPBF16)
        )
        in_maps.append({"kt": kt, "vt": vt, "qt": qt, **shared})
    return in_maps


def kernel(**inputs) -> np.ndarray:
    global LAST_EXEC_NS
    nc = get_nc()
    in_maps = make_in_maps(inputs)
    kwargs = {}
    if TRACE and TRACE_ALL_CORES:
        kwargs["trace_cores"] = list(range(8))
    res = run_bass_kernel_spmd(
        nc, in_maps, core_ids=list(range(8)), trace=TRACE, **kwargs
    )
    LAST_EXEC_NS = res.exec_time_ns

    out = np.empty((B, S, D), dtype=np.float32)
    for c in range(8):
        b, h = divmod(c, 2)
        out[b, h * SQ : (h + 1) * SQ, :] = res.results[c]["o"]
    return out


# revision 15
# speedup vs baseline: 1.3395x; 1.1256x over previous
"""Trainium2 Bass kernel for single-head attention (nn_MultiHeadAttention).

Reference computation (B=4, S=2048, D=1024, fp32):
    K = _K @ Wk.T + bk ; V = _V @ Wv.T + bv ; Q = _Q @ Wq.T + bq
    scores[b,k,q] = (K[b,k,:] . Q[b,q,:]) / sqrt(D)
    alpha = softmax(scores, axis=keys)
    V_[b,q,:] = sum_k V[b,k,:] * alpha[b,k,q]
    O = V_ @ Wo.T + bo

Sharding: core c = (b, h) with b = c//2 (batch), h = c%2 (query half of
1024). Each core handles the full key/value sequence of its batch and a
1024-query slice. Cores {2b, 2b+1} share batch b: each projects half the
keys/values and the halves are exchanged pair-wise with AllGathers.

Structural choices (aimed at zero PE idle — the HAM clock gate
re-throttles the array from 2.4 to 1.2 GHz after ~3.4us idle — and at
keeping the DVE out of the PE's PSUM-ring critical path):
  - Own-half-in-place exchange: each core's projected half goes straight
    to its final SBUF slot (local key order = [own half; partner half];
    softmax and the V-weighted sum are key-order invariant so any local
    order is valid). The pair AllGather output holds [rank0; rank1]; the
    partner half is recovered rank-agnostically as (r0 - own) + r1 with
    two in-place bf16 DVE ops. One of r0/r1 is bit-identical to own, so
    the recovery is exact on those ranks and ~0.5% on the others — well
    inside budget. Phase B scores own-half keys while the collective is
    in flight; the recovery windows (post-collective) fall where the DVE
    is otherwise idle.
  - All input streams load as [128, 1024] tiles (2KB per partition line;
    small descriptors halve effective DMA bandwidth) into fresh slots
    emitted up front, so everything is resident before the collective
    window opens and nothing contends with the collective's HBM traffic.
    The input-stream pool is released before phase B and its region
    reused for the exp tiles.
  - DMA queue split: input streams + gather-back loads on the Sync DGE
    queue; K-stream, collective staging stores + output stores on the
    Activation DGE queue.
  - The V bias is folded host-side into the output bias (softmax weights
    sum to 1, so (V_+bv)@Wo.T+bo == V_@Wo.T + (bv@Wo.T+bo)); the V
    projection drains PSUM via ScalarE copies, keeping the DVE free.
  - A burst of dummy matmuls fed from the first weight block (no memset
    dependency — the Vector engine's start preamble is ~7us) warms the
    HAM clock gate before real work; a tiny AllGather issued first
    absorbs the one-time comm-init barrier. The V exchange is split in
    two so its halves land well before phase C needs them.
  - Key-sums for softmax: exp tiles accumulate on GpSimd (otherwise
    idle), reduced across partitions at the end with one fp32
    ones-matmul per query chunk instead of 32 per-block ones-matmuls.
All main matmuls are bf16 (M=128, N=512) accumulating in fp32 PSUM.
"""

import sys

if "/opt/trn_rl_repo" not in sys.path:
    sys.path.insert(0, "/opt/trn_rl_repo")

import ml_dtypes
import numpy as np

import concourse.bass as bass
import concourse.tile as tile
from concourse import bacc, mybir
from concourse.bass_utils import run_bass_kernel_spmd

B, S, D = 4, 2048, 1024
SQ = 1024  # queries per core
SH = 1024  # keys/values projected per core (pair exchange fills the rest)
P = 128  # partitions
CH = 512  # matmul moving free dim (one fp32 PSUM bank)
EB = D // P  # 8 feature blocks
DB = D // P  # 8 contraction blocks
KB = S // P  # 16 key blocks
QB = SQ // P  # 8 query blocks
QC = SQ // CH  # 2 query chunks
FC = D // CH  # 2 output-feature chunks
SCALE = 1.0 / np.sqrt(np.float32(D))  # folded into exp()

F32 = mybir.dt.float32
BF16 = mybir.dt.bfloat16
AF = mybir.ActivationFunctionType
NPBF16 = ml_dtypes.bfloat16

N_WARM_MM = 16  # dummy matmuls to flip the HAM clock gate before real work

# test.py can flip this to get a profiled run; the measured NEFF time (max
# over traced cores) lands in LAST_EXEC_NS.
TRACE = False
TRACE_ALL_CORES = False
LAST_EXEC_NS = None

_NC_CACHE = None


def _build_nc() -> bass.Bass:
    # Bacc (not plain Bass): its finalize() pipeline splits multi-sem waits
    # into event-semaphore chains — TRN2 instructions take at most 1 wait.
    nc = bacc.Bacc(num_devices=8)

    kt_d = nc.dram_tensor("kt", [D, SH], BF16, kind="ExternalInput")
    vt_d = nc.dram_tensor("vt", [D, SH], BF16, kind="ExternalInput")
    qt_d = nc.dram_tensor("qt", [D, SQ], BF16, kind="ExternalInput")
    wkt_d = nc.dram_tensor("wkt", [D, D], BF16, kind="ExternalInput")
    wqt_d = nc.dram_tensor("wqt", [D, D], BF16, kind="ExternalInput")
    wvt_d = nc.dram_tensor("wvt", [D, D], BF16, kind="ExternalInput")
    wot_d = nc.dram_tensor("wot", [D, D], BF16, kind="ExternalInput")
    bk_d = nc.dram_tensor("bk", [P, EB], F32, kind="ExternalInput")
    bq_d = nc.dram_tensor("bq", [P, EB], F32, kind="ExternalInput")
    bob_d = nc.dram_tensor("bob", [P, D], BF16, kind="ExternalInput")
    o_d = nc.dram_tensor("o", [SQ, D], F32, kind="ExternalOutput")

    with tile.TileContext(nc) as tc:
        # Pools are stack-allocated per SBUF side. The weight ring (wa,
        # bufs=2) cycles wkt->wqt->wvt->wot through two 16KB slots; each
        # reuse WAR-waits only on a prior phase's matmuls, done long
        # before the next weight is needed. xs (input streams) is top of
        # the right stack so it can be released before phase B and its
        # region recycled for the exp tiles.
        p_misc = tc.alloc_tile_pool(name="misc", bufs=1, side="left")
        p_ps = tc.alloc_tile_pool(name="ps", bufs=6, space="PSUM")
        p_pss = tc.alloc_tile_pool(name="pss", bufs=2, space="PSUM")
        p_v = tc.alloc_tile_pool(name="v", bufs=1, side="right")
        p_vs = tc.alloc_tile_pool(name="vs", bufs=8, side="right")
        p_cs = tc.alloc_tile_pool(name="cs", bufs=4, side="right")
        p_xs = tc.alloc_tile_pool(name="xs", bufs=16, side="right")
        p_wa = tc.alloc_tile_pool(name="wa", bufs=2, side="left")
        p_kqt = tc.alloc_tile_pool(name="kqt", bufs=1, side="left")

        p_dram = tc.alloc_tile_pool(name="dram", bufs=1, space="DRAM")

        dma = nc.sync.dma_start
        dma_act = nc.scalar.dma_start

        recip_sb = p_misc.tile([P, SQ], F32)
        ones32_sb = p_misc.tile([P, P], F32)  # key-sum partition reduction
        acc_sb = p_misc.tile([P, QC, CH], F32)  # exp-sum accumulators
        nc.vector.memset(ones32_sb[:], 1.0)

        # Pair-wise exchange groups: {2b, 2b+1} share batch b.
        # The first collective pays a large one-time comm-init cost, so a
        # 128-byte warmup AllGather is issued immediately (staged via the
        # fast Sync DGE — the gpsimd software DGE takes ~10us to build
        # descriptors) and initializes the channels during phase A.
        CC_GROUPS = [[0, 1], [2, 3], [4, 5], [6, 7]]
        warm_in = p_dram.tile([1, 64], BF16)
        warm_out = p_dram.tile([2, 64], BF16)
        dma(out=warm_in[:], in_=kt_d[0:1, 0:64])
        nc.gpsimd.collective_compute(
            "AllGather",
            mybir.AluOpType.bypass,
            replica_groups=CC_GROUPS,
            ins=[warm_in.opt()],
            outs=[warm_out.opt()],
        )
        cc_kin = p_dram.tile([D, SH], BF16)
        cc_kout = p_dram.tile([2 * D, SH], BF16)
        cc_vin = p_dram.tile([SH, D], BF16)
        cc_vouta = p_dram.tile([SH, D], BF16)  # [r0 kb0-3; r1 kb0-3]
        cc_voutb = p_dram.tile([SH, D], BF16)  # [r0 kb4-7; r1 kb4-7]

        # One DMA per d-block so loads spread across HW queues and each
        # matmul depends only on its own 256KB slice.
        def load_w(name):
            t = p_wa.tile([P, DB, D], BF16, tag="w", name=name)
            src = {"wkt": wkt_d, "wqt": wqt_d, "wvt": wvt_d, "wot": wot_d}[
                name
            ].rearrange("(a p) e -> p a e", p=P)
            for a in range(DB):
                dma(out=t[:, a, :], in_=src[:, a, :])
            return t

        wkt_sb = load_w("wkt")
        bk_sb = p_misc.tile([P, EB], F32)
        dma(out=bk_sb[:], in_=bk_d[:])
        bq_sb = p_misc.tile([P, EB], F32)
        dma(out=bq_sb[:], in_=bq_d[:])

        # ---- PE warmup: flip the HAM clock gate to 2.4 GHz while the
        # input DMAs are still in flight. The dummy matmuls read the first
        # weight block (lands ~3us after the engine preambles — a memset
        # operand would be gated on the Vector engine's ~7us start
        # preamble) and write a PSUM slot recycled by the projections.
        warm_ps = p_ps.tile([P, CH], F32, tag="ps", name="ps")
        for _ in range(N_WARM_MM):
            nc.tensor.matmul(
                warm_ps[:, 0:P],
                wkt_sb[:, 0, 0:P],
                wkt_sb[:, 0, 0:P],
                start=True,
                stop=True,
            )

        kt_sb = p_kqt.tile([P, EB, S], BF16)  # K.T: [e_p, e_blk, k]
        qt_sb = p_kqt.tile([P, EB, SQ], BF16)  # Q.T: [e_p, e_blk, q]
        v_sb = p_v.tile([P, KB, D], BF16)  # V:   [k_p, k_blk, e]

        # ---- Phase A: projections ----
        # Q.T and K.T: out[e, s] = sum_d W.T[d, e] (stationary) @ _X.T[d, s]
        # Input streams are [128, 1024] tiles; matmuls slice the moving
        # operand per 512-chunk. The K stream rides the Activation DGE
        # queue so it lands in parallel with the weights on the Sync queue.
        def kq_proj(proj_w, proj_in, proj_out, proj_b, nchunk, dma_x=dma):
            xtt = []
            for d_ in range(DB):
                t = p_xs.tile([P, SH], BF16, tag="xtt", name="xtt")
                dma_x(out=t[:], in_=proj_in[d_ * P : (d_ + 1) * P, :])
                xtt.append(t)
            for sc in range(nchunk):
                for eb in range(EB):
                    ps = p_ps.tile([P, CH], F32, tag="ps", name="ps")
                    for d_ in range(DB):
                        nc.tensor.matmul(
                            ps[:],
                            proj_w[:, d_, eb * P : (eb + 1) * P],
                            xtt[d_][:, sc * CH : (sc + 1) * CH],
                            start=(d_ == 0),
                            stop=(d_ == DB - 1),
                        )
                    # DVE, not ACT: ~3x faster per copy-out, frees the psum
                    # slot sooner, and keeps ScalarE clear for phase B's exp
                    nc.vector.tensor_scalar_add(
                        proj_out[:, eb, sc * CH : (sc + 1) * CH],
                        ps[:],
                        proj_b[:, eb : eb + 1],
                    )

        # K.T own half lands directly in its final slot kt_sb[:, :, 0:SH].
        kq_proj(wkt_sb, kt_d, kt_sb, bk_sb, SH // CH, dma_x=dma_act)
        for eb in range(EB):
            dma_act(out=cc_kin[eb * P : (eb + 1) * P, :], in_=kt_sb[:, eb, 0:SH])
        nc.gpsimd.collective_compute(
            "AllGather",
            mybir.AluOpType.bypass,
            replica_groups=CC_GROUPS,
            ins=[cc_kin.opt()],
            outs=[cc_kout.opt()],
        )

        wqt_sb = load_w("wqt")
        kq_proj(wqt_sb, qt_d, qt_sb, bq_sb, QC)

        wvt_sb = load_w("wvt")
        vtb = []
        for d_ in range(DB):
            t = p_vs.tile([P, SH], BF16, tag="vtt", name="vtt")
            dma(out=t[:], in_=vt_d[d_ * P : (d_ + 1) * P, :])
            vtb.append(t)

        # V natural: out[k, e] = sum_d _V.T[d, k] (stationary) @ Wv.T[d, e].
        # The V bias is folded into the output bias host-side, so the PSUM
        # drain is a plain ScalarE copy — the DVE stays free for the
        # partner-recovery windows. The V exchange goes out in two halves
        # so partner values land well before phase C.
        for kb in range(SH // P):
            pse = [p_ps.tile([P, CH], F32, tag="ps", name="ps") for _ in range(FC)]
            for d_ in range(DB):
                for eh in range(FC):
                    nc.tensor.matmul(
                        pse[eh][:],
                        vtb[d_][:, kb * P : (kb + 1) * P],
                        wvt_sb[:, d_, eh * CH : (eh + 1) * CH],
                        start=(d_ == 0),
                        stop=(d_ == DB - 1),
                    )
            for eh in range(FC):
                nc.scalar.activation(
                    v_sb[:, kb, eh * CH : (eh + 1) * CH], pse[eh][:], AF.Copy
                )
            dma_act(out=cc_vin[kb * P : (kb + 1) * P, :], in_=v_sb[:, kb, :])
            if kb == 3:
                nc.gpsimd.collective_compute(
                    "AllGather",
                    mybir.AluOpType.bypass,
                    replica_groups=CC_GROUPS,
                    ins=[cc_vin[0 : SH // 2, :].opt()],
                    outs=[cc_vouta.opt()],
                )
        nc.gpsimd.collective_compute(
            "AllGather",
            mybir.AluOpType.bypass,
            replica_groups=CC_GROUPS,
            ins=[cc_vin[SH // 2 : SH, :].opt()],
            outs=[cc_voutb.opt()],
        )

        wot_sb = load_w("wot")
        bob_sb = p_misc.tile([P, D], BF16)
        dma(out=bob_sb[:], in_=bob_d[:])

        # Partner halves: partner = (r0 - own) + r1, computed in place in
        # bf16 (2x DVE rate). r0 lands directly in the partner slot; r1
        # stages through a small ring. Exact where own == r0; ~0.5%
        # element noise where own == r1 — inside budget.
        for eb in range(EB):
            dst = kt_sb[:, eb, SH:S]
            dma(out=dst, in_=cc_kout[eb * P : (eb + 1) * P, :])
            r1 = p_cs.tile([P, SH], BF16, tag="cs", name="cs")
            dma(out=r1[:], in_=cc_kout[D + eb * P : D + (eb + 1) * P, :])
            nc.vector.tensor_sub(dst, dst, kt_sb[:, eb, 0:SH])
            nc.vector.tensor_add(dst, dst, r1[:])
        for kb in range(SH // P):
            half, row = (cc_vouta, kb * P) if kb < 4 else (cc_voutb, (kb - 4) * P)
            dst = v_sb[:, 8 + kb, :]
            dma(out=dst, in_=half[row : row + P, :])
            r1 = p_cs.tile([P, SH], BF16, tag="cs", name="cs")
            dma(out=r1[:], in_=half[SH // 2 + row : SH // 2 + row + P, :])
            nc.vector.tensor_sub(dst, dst, v_sb[:, kb, :])
            nc.vector.tensor_add(dst, dst, r1[:])

        p_xs.release()
        p_es = tc.alloc_tile_pool(name="es", bufs=1, side="right")
        es_sb = p_es.tile([P, KB, SQ], BF16)  # exp(scores): [k_p, k_blk, q]

        # ---- Phase B: scores[k, q] = K.T' @ Q.T, exp, and key-sums ----
        # Key blocks 0..7 are the own half (ready right after the K
        # projection); 8..15 are the partner half (recovered well before
        # the PE's in-order queue reaches them). exp tiles accumulate on
        # GpSimd; the partition reduction happens once at the end with a
        # single fp32 ones-matmul per query chunk.
        nc.gpsimd.memset(acc_sb[:], 0.0)
        for kb in range(KB):
            psq = [p_ps.tile([P, CH], F32, tag="ps", name="ps") for _ in range(QC)]
            for eb in range(EB):
                for qc in range(QC):
                    nc.tensor.matmul(
                        psq[qc][:],
                        kt_sb[:, eb, kb * P : (kb + 1) * P],
                        qt_sb[:, eb, qc * CH : (qc + 1) * CH],
                        start=(eb == 0),
                        stop=(eb == EB - 1),
                    )
            for qc in range(QC):
                nc.scalar.activation(
                    es_sb[:, kb, qc * CH : (qc + 1) * CH],
                    psq[qc][:],
                    AF.Exp,
                    scale=float(SCALE),
                )
                nc.gpsimd.tensor_add(
                    acc_sb[:, qc, :],
                    acc_sb[:, qc, :],
                    es_sb[:, kb, qc * CH : (qc + 1) * CH],
                )

        p_kqt.release()
        p_vtu = tc.alloc_tile_pool(name="vtu", bufs=1, side="left")
        vtu_sb = p_vtu.tile([P, EB, SQ], BF16)  # normalized V_.T: [e_p, e_blk, q]

        # ---- Phase C: V_.T[e, q] = (sum_k V[k, e] es[k, q]) * recip[q] ----
        # The key-sum reduction (s_ps) and reciprocal are emitted after
        # eb=0's matmul group: their inputs are ready at phase-B end, so
        # the PE never stalls on the exp->accumulate tail, and the recip
        # lands on the DVE before eb=0's normalization multiplies need it.
        for eb in range(EB):
            psq = [p_ps.tile([P, CH], F32, tag="ps", name="ps") for _ in range(QC)]
            for kb in range(KB):
                for qc in range(QC):
                    nc.tensor.matmul(
                        psq[qc][:],
                        v_sb[:, kb, eb * P : (eb + 1) * P],
                        es_sb[:, kb, qc * CH : (qc + 1) * CH],
                        start=(kb == 0),
                        stop=(kb == KB - 1),
                    )
            if eb == 0:
                for qc in range(QC):
                    sp = p_pss.tile([P, CH], F32, tag="sps", name="s_ps")
                    nc.tensor.matmul(
                        sp[:], ones32_sb[:], acc_sb[:, qc, :], start=True, stop=True
                    )
                    nc.vector.reciprocal(recip_sb[:, qc * CH : (qc + 1) * CH], sp[:])
            for qc in range(QC):
                nc.vector.tensor_mul(
                    vtu_sb[:, eb, qc * CH : (qc + 1) * CH],
                    psq[qc][:],
                    recip_sb[:, qc * CH : (qc + 1) * CH],
                )

        p_o = tc.alloc_tile_pool(name="o", bufs=3, side="left")

        # ---- Phase D: O[q, f] = V_.T' @ Wo.T + bo' ----
        for qb in range(QB):
            ot = p_o.tile([P, D], F32, tag="ot", name="ot")
            for fc in range(FC):
                ps = p_ps.tile([P, CH], F32, tag="ps", name="ps")
                for eb in range(EB):
                    nc.tensor.matmul(
                        ps[:],
                        vtu_sb[:, eb, qb * P : (qb + 1) * P],
                        wot_sb[:, eb, fc * CH : (fc + 1) * CH],
                        start=(eb == 0),
                        stop=(eb == EB - 1),
                    )
                nc.vector.tensor_add(
                    ot[:, fc * CH : (fc + 1) * CH],
                    ps[:],
                    bob_sb[:, fc * CH : (fc + 1) * CH],
                )
            # per-chunk stores so the first half ships while the second
            # half's add is still running
            for fc in range(FC):
                dma_act(
                    out=o_d[qb * P : (qb + 1) * P, fc * CH : (fc + 1) * CH],
                    in_=ot[:, fc * CH : (fc + 1) * CH],
                )

        p_es.release()
        p_cs.release()
        p_vs.release()
        p_v.release()
        p_o.release()
        p_vtu.release()
        p_wa.release()
        p_misc.release()
        p_dram.release()
        p_pss.release()
        p_ps.release()

    nc.finalize()
    return nc


def get_nc() -> bass.Bass:
    global _NC_CACHE
    if _NC_CACHE is None:
        _NC_CACHE = _build_nc()
    return _NC_CACHE


def make_in_maps(inputs: dict) -> list[dict]:
    _K = np.asarray(inputs["_K"], dtype=np.float32)
    _V = np.asarray(inputs["_V"], dtype=np.float32)
    _Q = np.asarray(inputs["_Q"], dtype=np.float32)
    wo = np.asarray(inputs["Wo"], np.float32)
    # V-bias folding: softmax weights sum to 1, so
    # (V_+bv)@Wo.T + bo == V_@Wo.T + (bv@Wo.T + bo).
    bo_eff = np.asarray(inputs["bo"], np.float32) + np.asarray(
        inputs["bv"], np.float32
    ) @ wo.T

    shared = {
        "wkt": np.ascontiguousarray(
            np.asarray(inputs["Wk"], np.float32).T.astype(NPBF16)
        ),
        "wqt": np.ascontiguousarray(
            np.asarray(inputs["Wq"], np.float32).T.astype(NPBF16)
        ),
        "wvt": np.ascontiguousarray(
            np.asarray(inputs["Wv"], np.float32).T.astype(NPBF16)
        ),
        "wot": np.ascontiguousarray(wo.T.astype(NPBF16)),
        "bk": np.ascontiguousarray(
            np.asarray(inputs["bk"], np.float32).reshape(EB, P).T
        ),
        "bq": np.ascontiguousarray(
            np.asarray(inputs["bq"], np.float32).reshape(EB, P).T
        ),
        "bob": np.ascontiguousarray(
            np.broadcast_to(bo_eff.astype(NPBF16), (P, D))
        ),
    }

    in_maps = []
    for c in range(8):
        b, h = divmod(c, 2)
        # Each core projects its own key/value half; the pair AllGather +
        # on-chip recovery fills the partner half. Local key order is
        # [own half; partner half] — valid because softmax and the
        # V-weighted sum are key-order invariant.
        kt = np.ascontiguousarray(
            _K[b, h * SH : (h + 1) * SH, :].T.astype(NPBF16)
        )
        vt = np.ascontiguousarray(
            _V[b, h * SH : (h + 1) * SH, :].T.astype(NPBF16)
        )
        qt = np.ascontiguousarray(
            _Q[b, h * SQ : (h + 1) * SQ, :].T.astype(NPBF16)
        )
        in_maps.append({"kt": kt, "vt": vt, "qt": qt, **shared})
    return in_maps


def kernel(**inputs) -> np.ndarray:
    global LAST_EXEC_NS
    nc = get_nc()
    in_maps = make_in_maps(inputs)
    kwargs = {}
    if TRACE and TRACE_ALL_CORES:
        kwargs["trace_cores"] = list(range(8))
    res = run_bass_kernel_spmd(
        nc, in_maps, core_ids=list(range(8)), trace=TRACE, **kwargs
    )
    LAST_EXEC_NS = res.exec_time_ns

    out = np.empty((B, S, D), dtype=np.float32)
    for c in range(8):
        b, h = divmod(c, 2)
        out[b, h * SQ : (h + 1) * SQ, :] = res.results[c]["o"]
    return out
